# revision 3
# baseline (speedup 1.0000x reference)
"""Trainium2 Bass kernel for nn_GCNModel (2-layer UI GCN + 2-layer social GCN).

Self-contained: host-side sharding/preprocessing + Bass/Tile program +
PJRT SPMD execution on 8 NeuronCores.

Strategy (dest-sharded 1D graph partition, 8 cores):
  - Each core owns a contiguous range of destination rows for both branches.
  - spmm computed as per-chunk matmuls: zT[j, s] += sum_e G[e, j] * O[e, s]
    where G holds gathered source rows (128 edges per chunk) and O is a
    host-built one-hot-times-value selection matrix mapping each edge to its
    destination slot within an S-row window.
  - Layer 1 tables (concat(user,item) / user_emb) are model inputs, so the
    per-edge "gather" is precomputed on the host into a sequential stream.
  - Layer 1 outputs are AllGathered (fp16) so layer 2 can gather from the
    full table with per-chunk indirect DMAs (128 int32 offsets each).
  - (A @ prev) @ W == A @ (prev @ W): the 64x64 weight is applied to the
    128-row spmm output blocks (transposed layout makes this a single
    matmul), then LeakyReLU(0.5), L2-normalize, and accumulate.
"""
import numpy as np

import jax
from jax.sharding import Mesh, PartitionSpec
from jax.experimental.shard_map import shard_map

import concourse.bass as bass
import concourse.bacc as bacc
import concourse.mybir as mybir
import concourse.tile as tile
from concourse import bass2jax

F16 = mybir.dt.float16
F32 = mybir.dt.float32
I32 = mybir.dt.int32

N_USER, N_ITEM, H = 100000, 150000, 64
N_NODES = N_USER + N_ITEM
NCORES = 8
SLOPE = 0.5

# ----------------------------------------------------------------------------
# host-side preprocessing
# ----------------------------------------------------------------------------


def _prep_branch(rows, cols, vals, n_dest, n_cores, S, table16=None):
    """Shard edges by destination range, sort by dest, pack into 128-edge
    chunks aligned to S-row destination blocks (uniform chunk counts across
    cores so one SPMD program fits all cores).

    Returns (meta, per_core) where per_core[c] has:
      O:    [128, C, S]  f16   selection matrices (val at dest slot)
      G:    [128, C, H]  f16   host-gathered source rows (if table16 given)
      offs: [128, C]     i32   source row ids (if table16 is None)
    """
    R = n_dest // n_cores
    Rp = ((R + 127) // 128) * 128
    NBLK = Rp // S
    core = rows // R
    np.minimum(core, n_cores - 1, out=core)
    rel = rows - core * R

    per_core_edges = []
    bcs = np.zeros((n_cores, NBLK), np.int64)
    for c in range(n_cores):
        sel = core == c
        rel_c = rel[sel]
        order = np.argsort(rel_c, kind="stable")
        rel_s = rel_c[order]
        col_s = cols[sel][order]
        val_s = vals[sel][order]
        blk = rel_s // S
        bcs[c] = np.bincount(blk, minlength=NBLK)
        per_core_edges.append((rel_s, col_s, val_s, blk))

    cpb = np.maximum(1, (bcs.max(axis=0) + 127) // 128).astype(np.int64)
    C = int(cpb.sum())
    # pad chunk count to a multiple of 16 (stream-load group size)
    C_pad = ((C + 15) // 16) * 16
    cpb[-1] += C_pad - C
    chunk_base = np.concatenate([[0], np.cumsum(cpb)]).astype(np.int64)

    per_core = []
    for c in range(n_cores):
        rel_s, col_s, val_s, blk = per_core_edges[c]
        E_c = len(rel_s)
        block_start = np.concatenate([[0], np.cumsum(bcs[c])]).astype(np.int64)
        rank = np.arange(E_c, dtype=np.int64) - block_start[blk]
        chunk = chunk_base[blk] + (rank >> 7)
        pos = rank & 127
        slot = rel_s - blk * S
        O = np.zeros((128, C_pad, S), np.float16)
        O[pos, chunk, slot] = val_s.astype(np.float16)
        entry = {"O": O}
        if table16 is not None:
            G = np.zeros((128, C_pad, H), np.float16)
            G[pos, chunk] = table16[col_s]
            entry["G"] = G
        else:
            offs = np.zeros((128, C_pad), np.int32)
            offs[pos, chunk] = col_s.astype(np.int32)
            entry["offs"] = offs
        per_core.append(entry)

    meta = {"S": S, "NBLK": NBLK, "C": C_pad, "cpb": cpb,
            "chunk_base": chunk_base, "R": R, "Rp": Rp}
    return meta, per_core


def _pack_rows(x, Rp):
    """[R, H] f32 -> [128, Rp//128 * H] in (partition, block, H) layout."""
    R = x.shape[0]
    buf = np.zeros((Rp, H), np.float32)
    buf[:R] = x
    return np.ascontiguousarray(
        buf.reshape(Rp // 128, 128, H).transpose(1, 0, 2)
    ).reshape(128, -1)


def _unpack_rows(y, R):
    """[128, NB*H] -> [R, H]."""
    NB = y.shape[1] // H
    return y.reshape(128, NB, H).transpose(1, 0, 2).reshape(NB * 128, H)[:R]


# ----------------------------------------------------------------------------
# device program
# ----------------------------------------------------------------------------


def _emit_identity(nc, pool):
    """64x64 f32 identity via iota + compare (standard-lib ops only)."""
    a = pool.tile([64, 64], I32)
    b = pool.tile([64, 64], I32)
    ident = pool.tile([64, 64], F32)
    nc.gpsimd.iota(a[:], pattern=[[1, 64]], base=0, channel_multiplier=0)
    nc.gpsimd.iota(b[:], pattern=[[0, 64]], base=0, channel_multiplier=1)
    nc.vector.tensor_tensor(out=ident[:], in0=a[:], in1=b[:],
                            op=mybir.AluOpType.is_equal)
    return ident


def _emit_layer(nc, tc, P, meta, *, table=None, g_stream=None, o_stream,
                offs_stream=None, w_tile, acc_tile, identity,
                ag_in=None, rows_exact=None):
    """Emit one GCN layer for this core's destination shard.

    table+offs_stream: layer-2 style (indirect gathers from `table`)
    g_stream: layer-1 style (host-gathered rows streamed from DRAM)
    """
    S = meta["S"]
    NBLK = meta["NBLK"]
    C = meta["C"]
    cpb = meta["cpb"]
    chunk_base = meta["chunk_base"]
    Rp = meta["Rp"]
    wps = 512 // S          # S-blocks per 512-row tail group
    n_tg = (NBLK + wps - 1) // wps
    GRP = 16                # chunks per stream-load group

    offs_t = None
    if offs_stream is not None:
        offs_t = P["offs"].tile([128, C], I32)
        nc.sync.dma_start(out=offs_t[:], in_=offs_stream[:])

    cur_g = -1
    G_t = O_t = None
    for tg in range(n_tg):
        b0 = tg * wps
        nb = min(wps, NBLK - b0)
        width = nb * S
        zT = P["pz"].tile([64, 512], F32, space="PSUM", tag="zT")
        for k in range(nb):
            b = b0 + k
            nch = int(cpb[b])
            for j in range(nch):
                c = int(chunk_base[b]) + j
                g = c // GRP
                if g != cur_g:
                    if g_stream is not None:
                        G_t = P["pg"].tile([128, GRP, H], F16, tag="G")
                        nc.sync.dma_start(
                            out=G_t[:], in_=g_stream[:, g * GRP:(g + 1) * GRP, :])
                    O_t = P["po"].tile([128, GRP, S], F16, tag="O")
                    nc.sync.dma_start(
                        out=O_t[:], in_=o_stream[:, g * GRP:(g + 1) * GRP, :])
                    cur_g = g
                if g_stream is not None:
                    lhsT = G_t[:, c % GRP, :]
                else:
                    G2 = P["pg2"].tile([128, H], F16, tag="G2")
                    nc.gpsimd.indirect_dma_start(
                        out=G2[:], out_offset=None, in_=table[:],
                        in_offset=bass.IndirectOffsetOnAxis(
                            ap=offs_t[:, c:c + 1], axis=0))
                    lhsT = G2[:]
                nc.tensor.matmul(
                    out=zT[:, k * S:(k + 1) * S], lhsT=lhsT,
                    rhs=O_t[:, c % GRP, :],
                    start=(j == 0), stop=(j == nch - 1))
        # ---- tail: W, leaky, transpose, l2norm, accumulate ----
        zs = P["pzs"].tile([64, 512], F16, tag="zs")
        nc.vector.tensor_copy(out=zs[:, :width], in_=zT[:, :width])
        tT = P["pt"].tile([64, 512], F32, space="PSUM", tag="tT")
        nc.tensor.matmul(out=tT[:, :width], lhsT=w_tile[:], rhs=zs[:, :width],
                         start=True, stop=True)
        u = P["pu"].tile([64, 512], F32, tag="u")
        nc.vector.tensor_scalar(out=u[:, :width], in0=tT[:, :width],
                                scalar1=SLOPE, scalar2=None,
                                op0=mybir.AluOpType.mult)
        nT = P["pn"].tile([64, 512], F32, tag="nT")
        nc.vector.tensor_tensor(out=nT[:, :width], in0=tT[:, :width],
                                in1=u[:, :width], op=mybir.AluOpType.max)
        nb2 = (width + 127) // 128
        ntp = P["pnp"].tile([128, 4, H], F32, space="PSUM", tag="ntp")
        for t in range(nb2):
            w2 = min(128, width - t * 128)
            nc.tensor.transpose(out=ntp[:w2, t, :],
                                in_=nT[:, t * 128:t * 128 + w2],
                                identity=identity[:])
        sq = P["psq"].tile([128, 4, H], F32, tag="sq")
        nc.scalar.activation(out=sq[:, :nb2, :], in_=ntp[:, :nb2, :],
                             func=mybir.ActivationFunctionType.Square)
        ss = P["pss"].tile([128, 4], F32, tag="ss")
        nc.vector.reduce_sum(out=ss[:, :nb2], in_=sq[:, :nb2, :],
                             axis=mybir.AxisListType.X)
        nrm = P["pnr"].tile([128, 4], F32, tag="nrm")
        nc.scalar.activation(out=nrm[:, :nb2], in_=ss[:, :nb2],
                             func=mybir.ActivationFunctionType.Sqrt,
                             bias=P["epsb"][:])
        inv = P["pin"].tile([128, 4], F32, tag="inv")
        nc.vector.reciprocal(out=inv[:, :nb2], in_=nrm[:, :nb2])
        l2 = P["pl2"].tile([128, 4, H], F32, tag="l2")
        nc.vector.tensor_tensor(out=l2[:, :nb2, :], in0=ntp[:, :nb2, :],
                                in1=inv[:, :nb2].to_broadcast([128, nb2, H]),
                                op=mybir.AluOpType.mult)
        ab = 4 * tg
        nc.vector.tensor_add(
            out=acc_tile[:, ab:ab + nb2, :], in0=acc_tile[:, ab:ab + nb2, :],
            in1=l2[:, :nb2, :])
        if ag_in is not None:
            f16c = P["pf"].tile([128, 4, H], F16, tag="f16c")
            nc.scalar.copy(out=f16c[:, :nb2, :], in_=ntp[:, :nb2, :])
            for t in range(nb2):
                r0 = tg * 512 + t * 128
                rcount = min(128, rows_exact - r0)
                if rcount <= 0:
                    break
                nc.sync.dma_start(out=ag_in[r0:r0 + rcount, :],
                                  in_=f16c[:rcount, t, :])


def build_program(mu1, mu2, ms1, ms2, dims):
    """Build the full 4-phase SPMD program. m*: prep metas."""
    n_nodes = dims["n_nodes"]
    n_user = dims["n_user"]
    n_cores = dims["n_cores"]

    nc = bacc.Bacc("TRN2", target_bir_lowering=False, debug=False,
                   num_devices=n_cores)

    def din(name, shape, dt):
        return nc.dram_tensor(name, shape, dt, kind="ExternalInput")

    # per-core inputs
    g1_ui = din("g1_ui", [128, mu1["C"], H], F16)
    o1_ui = din("o1_ui", [128, mu1["C"], mu1["S"]], F16)
    o2_ui = din("o2_ui", [128, mu2["C"], mu2["S"]], F16)
    offs_ui = din("offs_ui", [128, mu2["C"]], I32)
    g1_s = din("g1_s", [128, ms1["C"], H], F16)
    o1_s = din("o1_s", [128, ms1["C"], ms1["S"]], F16)
    o2_s = din("o2_s", [128, ms2["C"], ms2["S"]], F16)
    offs_s = din("offs_s", [128, ms2["C"]], I32)
    w_ui = din("w_ui", [2, H, H], F16)
    w_s = din("w_s", [2, H, H], F16)
    e0_own = din("e0_own", [128, (mu1["Rp"] // 128) * H], F32)
    s0_own = din("s0_own", [128, (ms1["Rp"] // 128) * H], F32)

    out_ui = nc.dram_tensor("out_ui", [128, (mu1["Rp"] // 128) * H], F32,
                            kind="ExternalOutput")
    out_s = nc.dram_tensor("out_s", [128, (ms1["Rp"] // 128) * H], F32,
                           kind="ExternalOutput")

    ag_in_ui = nc.dram_tensor("ag_in_ui", [mu1["R"], H], F16)
    n1_table = nc.dram_tensor("n1_table", [n_nodes, H], F16,
                              addr_space="Shared")
    ag_in_s = nc.dram_tensor("ag_in_s", [ms1["R"], H], F16)
    m1_table = nc.dram_tensor("m1_table", [n_user, H], F16,
                              addr_space="Shared")

    groups = [list(range(n_cores))]

    with tile.TileContext(nc) as tc:
        P = {}
        pools = [
            ("pg", 3), ("po", 3), ("pg2", 12), ("offs", 1),
            ("pzs", 2), ("pu", 2), ("pn", 2), ("psq", 2), ("pss", 2),
            ("pnr", 2), ("pin", 2), ("pl2", 2), ("pf", 2),
            ("pacc", 1), ("pw", 1), ("pid", 1),
        ]
        import contextlib
        with contextlib.ExitStack() as stack:
            for name, bufs in pools:
                P[name] = stack.enter_context(tc.tile_pool(name=name, bufs=bufs))
            P["pz"] = stack.enter_context(
                tc.tile_pool(name="pz", bufs=2, space="PSUM"))
            P["pt"] = stack.enter_context(
                tc.tile_pool(name="pt", bufs=2, space="PSUM"))
            P["pnp"] = stack.enter_context(
                tc.tile_pool(name="pnp", bufs=2, space="PSUM"))

            identity = _emit_identity(nc, P["pid"])
            epsb = P["pid"].tile([128, 1], F32, tag="epsb")
            nc.vector.memset(epsb[:], 1e-12)
            P["epsb"] = epsb
            w_tiles = []
            for l in range(2):
                wt = P["pw"].tile([H, H], F16, tag=f"wui{l}")
                nc.sync.dma_start(out=wt[:], in_=w_ui[l])
                w_tiles.append(wt)
            ws_tiles = []
            for l in range(2):
                wt = P["pw"].tile([H, H], F16, tag=f"ws{l}")
                nc.sync.dma_start(out=wt[:], in_=w_s[l])
                ws_tiles.append(wt)

            acc_ui = P["pacc"].tile([128, mu1["Rp"] // 128, H], F32, tag="aui")
            nc.sync.dma_start(
                out=acc_ui[:],
                in_=e0_own[:].rearrange("p (b h) -> p b h", h=H))
            acc_s = P["pacc"].tile([128, ms1["Rp"] // 128, H], F32, tag="as")
            nc.sync.dma_start(
                out=acc_s[:],
                in_=s0_own[:].rearrange("p (b h) -> p b h", h=H))

            # phase 1: ui layer 1 (host-gathered stream)
            _emit_layer(nc, tc, P, mu1, g_stream=g1_ui, o_stream=o1_ui,
                        w_tile=w_tiles[0], acc_tile=acc_ui, identity=identity,
                        ag_in=ag_in_ui, rows_exact=mu1["R"])
            nc.gpsimd.collective_compute(
                "AllGather", mybir.AluOpType.bypass, replica_groups=groups,
                ins=[ag_in_ui[:]], outs=[n1_table[:]])

            # phase 2: social layer 1
            _emit_layer(nc, tc, P, ms1, g_stream=g1_s, o_stream=o1_s,
                        w_tile=ws_tiles[0], acc_tile=acc_s, identity=identity,
                        ag_in=ag_in_s, rows_exact=ms1["R"])
            nc.gpsimd.collective_compute(
                "AllGather", mybir.AluOpType.bypass, replica_groups=groups,
                ins=[ag_in_s[:]], outs=[m1_table[:]])

            # phase 3: ui layer 2 (indirect gathers from n1_table)
            _emit_layer(nc, tc, P, mu2, table=n1_table, o_stream=o2_ui,
                        offs_stream=offs_ui, w_tile=w_tiles[1],
                        acc_tile=acc_ui, identity=identity)

            # phase 4: social layer 2
            _emit_layer(nc, tc, P, ms2, table=m1_table, o_stream=o2_s,
                        offs_stream=offs_s, w_tile=ws_tiles[1],
                        acc_tile=acc_s, identity=identity)

            nc.sync.dma_start(
                out=out_ui[:],
                in_=acc_ui[:].rearrange("p b h -> p (b h)"))
            nc.sync.dma_start(
                out=out_s[:],
                in_=acc_s[:].rearrange("p b h -> p (b h)"))

    nc.compile()
    return nc


# ----------------------------------------------------------------------------
# PJRT runner (compile once, reuse)
# ----------------------------------------------------------------------------


class SpmdRunner:
    def __init__(self, nc, n_cores):
        bass2jax.install_neuronx_cc_hook()
        self.nc = nc
        self.n_cores = n_cores
        pname = nc.partition_id_tensor.name if nc.partition_id_tensor else None
        in_names, out_names, out_avals, zero_outs = [], [], [], []
        for alloc in nc.m.functions[0].allocations:
            if not isinstance(alloc, mybir.MemoryLocationSet):
                continue
            name = alloc.memorylocations[0].name
            if alloc.kind == "ExternalInput":
                if name != pname:
                    in_names.append(name)
            elif alloc.kind == "ExternalOutput":
                out_names.append(name)
                shape = tuple(alloc.tensor_shape)
                dtype = mybir.dt.np(alloc.dtype)
                out_avals.append(jax.core.ShapedArray(shape, dtype))
                zero_outs.append(np.zeros(shape, dtype))
        self.in_names, self.out_names = in_names, out_names
        self.out_avals, self.zero_outs = out_avals, zero_outs
        n_params = len(in_names)
        all_in = list(in_names) + list(out_names)
        if pname is not None:
            all_in.append(pname)
        donate = tuple(range(n_params, n_params + len(out_names)))

        def _body(*args):
            operands = list(args)
            if pname is not None:
                operands.append(bass2jax.partition_id_tensor())
            outs = bass2jax._bass_exec_p.bind(
                *operands, out_avals=tuple(out_avals),
                in_names=tuple(all_in), out_names=tuple(out_names),
                lowering_input_output_aliases=(),
                sim_require_finite=True, sim_require_nnan=True, nc=nc)
            return tuple(outs)

        devices = jax.devices()[:n_cores]
        self.mesh = Mesh(np.asarray(devices), ("core",))
        self.sharded = jax.jit(
            shard_map(_body, mesh=self.mesh,
                      in_specs=(PartitionSpec("core"),) * (n_params + len(out_names)),
                      out_specs=(PartitionSpec("core"),) * len(out_names),
                      check_rep=False),
            donate_argnums=donate, keep_unused=True)
        self.in_sharding = jax.sharding.NamedSharding(
            self.mesh, PartitionSpec("core"))

    def stage_inputs(self, in_maps):
        return [
            jax.device_put(
                np.concatenate([np.asarray(m[name]) for m in in_maps], axis=0),
                self.in_sharding)
            for name in self.in_names
        ]

    def make_zeros(self):
        return [
            jax.device_put(
                np.zeros((self.n_cores * z.shape[0], *z.shape[1:]), z.dtype),
                self.in_sharding)
            for z in self.zero_outs
        ]

    def run(self, staged):
        out = self.sharded(*staged, *self.make_zeros())
        jax.block_until_ready(out)
        return out

    def outputs_to_maps(self, out_arrs):
        return [
            {name: np.asarray(out_arrs[i]).reshape(
                self.n_cores, *self.out_avals[i].shape)[c]
             for i, name in enumerate(self.out_names)}
            for c in range(self.n_cores)
        ]


# ----------------------------------------------------------------------------
# public entry point
# ----------------------------------------------------------------------------

_CACHE = {}


def _prepare(inputs):
    user_emb = np.asarray(inputs["user_emb"], np.float32)
    item_emb = np.asarray(inputs["item_emb"], np.float32)
    ui_w = np.asarray(inputs["ui_weights"], np.float32)
    s_w = np.asarray(inputs["social_weights"], np.float32)
    ui_rows = np.asarray(inputs["ui_rows"], np.int64)
    ui_cols = np.asarray(inputs["ui_cols"], np.int64)
    ui_vals = np.asarray(inputs["ui_vals"], np.float32)
    s_rows = np.asarray(inputs["social_rows"], np.int64)
    s_cols = np.asarray(inputs["social_cols"], np.int64)
    s_vals = np.asarray(inputs["social_vals"], np.float32)

    n_user, n_item = user_emb.shape[0], item_emb.shape[0]
    n_nodes = n_user + n_item
    e0 = np.concatenate([user_emb, item_emb], axis=0)
    e016 = e0.astype(np.float16)
    u016 = e016[:n_user]

    mu1, pc_u1 = _prep_branch(ui_rows, ui_cols, ui_vals, n_nodes, NCORES,
                              64, table16=e016)
    mu2, pc_u2 = _prep_branch(ui_rows, ui_cols, ui_vals, n_nodes, NCORES,
                              128, table16=None)
    ms1, pc_s1 = _prep_branch(s_rows, s_cols, s_vals, n_user, NCORES,
                              64, table16=u016)
    ms2, pc_s2 = _prep_branch(s_rows, s_cols, s_vals, n_user, NCORES,
                              128, table16=None)

    dims = {"n_nodes": n_nodes, "n_user": n_user, "n_cores": NCORES}
    in_maps = []
    for c in range(NCORES):
        R_ui, Rp_ui = mu1["R"], mu1["Rp"]
        R_s, Rp_s = ms1["R"], ms1["Rp"]
        in_maps.append({
            "g1_ui": pc_u1[c]["G"], "o1_ui": pc_u1[c]["O"],
            "o2_ui": pc_u2[c]["O"], "offs_ui": pc_u2[c]["offs"],
            "g1_s": pc_s1[c]["G"], "o1_s": pc_s1[c]["O"],
            "o2_s": pc_s2[c]["O"], "offs_s": pc_s2[c]["offs"],
            "w_ui": ui_w.astype(np.float16), "w_s": s_w.astype(np.float16),
            "e0_own": _pack_rows(e0[c * R_ui:(c + 1) * R_ui], Rp_ui),
            "s0_own": _pack_rows(user_emb[c * R_s:(c + 1) * R_s], Rp_s),
        })
    key = (mu1["C"], mu2["C"], ms1["C"], ms2["C"],
           tuple(mu1["cpb"]), tuple(mu2["cpb"]),
           tuple(ms1["cpb"]), tuple(ms2["cpb"]))
    return key, (mu1, mu2, ms1, ms2), dims, in_maps


def kernel(**inputs):
    key, metas, dims, in_maps = _prepare(inputs)
    if key not in _CACHE:
        nc = build_program(*metas, dims)
        _CACHE[key] = SpmdRunner(nc, dims["n_cores"])
    runner = _CACHE[key]
    staged = runner.stage_inputs(in_maps)
    out_arrs = runner.run(staged)
    maps = runner.outputs_to_maps(out_arrs)
    mu1, mu2, ms1, ms2 = metas
    ui_parts = [_unpack_rows(
        maps[c]["out_ui"].reshape(128, -1), mu1["R"]) for c in range(NCORES)]
    s_parts = [_unpack_rows(
        maps[c]["out_s"].reshape(128, -1), ms1["R"]) for c in range(NCORES)]
    return (np.concatenate(ui_parts, axis=0),
            np.concatenate(s_parts, axis=0))
